# revision 2
# baseline (speedup 1.0000x reference)
"""Trainium2 Bass kernel for nn_DMlp_46823733461564 (dense_mlp).

Computes: token-grid 3x3 masked-neighborhood gather (pixel-shuffle +
reflection-pad + masked unfold, algebraically reduced to a channel-
permuted shifted gather) followed by fc1(1600->1024) + exact GELU +
fc2(1024->576).

Sharding: data-parallel over (batch, image-half) -> 8 cores, 8192 tokens
each; fc weights replicated. The gather runs on-device as strided DMAs
from a host-prepared reflection-extended channel-major image; matmuls run
in fp32r (single-pass reduced-precision fp32) on the PE.
"""
import os
import sys

import numpy as np

_TRN_REPO = "/opt/trn_rl_repo"
if _TRN_REPO not in sys.path:
    sys.path.insert(0, _TRN_REPO)

B, HIMG, WIMG = 4, 128, 128
C = 64
L = 576           # C * 9
NTOK = HIMG * WIMG
HID = 1024
OUTF = 576
INF = 1600        # C * 25
N_CORES = 8
ROWS_PER_CORE = HIMG // 2          # 64 token rows
TOK_PER_CORE = ROWS_PER_CORE * WIMG  # 8192
TILE_ROWS = 4                      # image rows per token tile
TT = TILE_ROWS * WIMG              # 512 tokens per tile
N_TILES = ROWS_PER_CORE // TILE_ROWS  # 16
KC = 13                            # ceil(25/2) K-chunks of (up to) 128

_MASK = np.array([
    [1, 0, 0, 1, 0, 0, 1],
    [0, 1, 0, 1, 0, 1, 0],
    [0, 0, 1, 1, 1, 0, 0],
    [1, 1, 1, 1, 1, 1, 1],
    [0, 0, 1, 1, 1, 0, 0],
    [0, 1, 0, 1, 0, 1, 0],
    [1, 0, 0, 1, 0, 0, 1]], dtype=bool)
MASK_POS = [(i, j) for i in range(7) for j in range(7) if _MASK[i, j]]


def _dmap(d):
    if d <= 1:
        return -1, d + 1
    if d <= 4:
        return 0, d - 2
    return 1, d - 5


KPOS = []
for (_di, _dj) in MASK_POS:
    _dh, _r1 = _dmap(_di)
    _dw, _r2 = _dmap(_dj)
    KPOS.append((_dh, _dw, _r1 * 3 + _r2))


def _swap_map(a, b, which):
    ch = np.arange(L)
    c, rem = ch // 9, ch % 9
    r1, r2 = rem // 3, rem % 3
    r = r1 if which == 0 else r2
    rs = np.where(r == a, b, np.where(r == b, a, r))
    if which == 0:
        return c * 9 + rs * 3 + r2
    return c * 9 + r1 * 3 + rs


def _build_xe(x):
    """x: (B, NTOK, L) -> xe: (B, L, HIMG+2, WIMG+2) reflection-extended,
    channel-permuted borders."""
    xt = np.ascontiguousarray(x.transpose(0, 2, 1)).reshape(B, L, HIMG, WIMG)
    xe = np.empty((B, L, HIMG + 2, WIMG + 2), dtype=np.float32)
    xe[:, :, 1:-1, 1:-1] = xt
    xe[:, :, 0, 1:-1] = xt[:, _swap_map(1, 2, 0), 0, :]
    xe[:, :, -1, 1:-1] = xt[:, _swap_map(0, 1, 0), -1, :]
    xe[:, :, :, 0] = np.take(xe[:, :, :, 1], _swap_map(1, 2, 1), axis=1)
    xe[:, :, :, -1] = np.take(xe[:, :, :, -2], _swap_map(0, 1, 1), axis=1)
    return xe


_NC_CACHE = {}


def _build_bass():
    if "nc" in _NC_CACHE:
        return _NC_CACHE["nc"]
    import concourse.bass as bass
    import concourse.mybir as mybir
    from concourse.tile import TileContext

    f32 = mybir.dt.float32
    f32r = mybir.dt.float32r
    AF = mybir.ActivationFunctionType
    Alu = mybir.AluOpType

    nc = bass.Bass("TRN2", target_bir_lowering=False, debug=False)
    xe = nc.dram_tensor("xe", (L, ROWS_PER_CORE + 2, WIMG + 2), f32r,
                        kind="ExternalInput")
    w1p = nc.dram_tensor("w1p", (INF, HID), f32r, kind="ExternalInput")
    w2t = nc.dram_tensor("w2t", (HID, OUTF), f32r, kind="ExternalInput")
    b1rs = nc.dram_tensor("b1rs", (128, HID // 128), f32, kind="ExternalInput")
    b2bc = nc.dram_tensor("b2bc", (128, OUTF), f32, kind="ExternalInput")
    out = nc.dram_tensor("out", (TOK_PER_CORE, OUTF), f32,
                         kind="ExternalOutput")

    # view of xe with the q sub-pixel index split out: [q, c, rows, cols]
    xe_q = xe.rearrange("(c q) h w -> q c h w", q=9)

    with TileContext(nc) as tc:
        with (
            tc.tile_pool(name="wpool", bufs=1) as wpool,
            tc.tile_pool(name="fpool", bufs=2) as fpool,
            tc.tile_pool(name="hpool", bufs=2) as hpool,
            tc.tile_pool(name="opool", bufs=3) as opool,
            tc.tile_pool(name="ps1", bufs=2, space="PSUM") as ps1,
            tc.tile_pool(name="ps2", bufs=2, space="PSUM") as ps2,
        ):
            # --- replicated weights, loaded once ---
            w1sb = []
            for j in range(KC):
                kr = min(128, INF - j * 128)
                t = wpool.tile([kr, HID], f32r, tag=f"w1_{j}")
                nc.sync.dma_start(out=t[:, :], in_=w1p[j * 128 : j * 128 + kr, :])
                w1sb.append(t)
            w2sb = []
            for j in range(HID // 128):
                t = wpool.tile([128, OUTF], f32r, tag=f"w2_{j}")
                nc.sync.dma_start(out=t[:, :], in_=w2t[j * 128 : (j + 1) * 128, :])
                w2sb.append(t)
            b1t = wpool.tile([128, HID // 128], f32, tag="b1")
            nc.sync.dma_start(out=b1t[:, :], in_=b1rs[:, :])
            b2t = wpool.tile([128, OUTF], f32, tag="b2")
            nc.sync.dma_start(out=b2t[:, :], in_=b2bc[:, :])

            for t_i in range(N_TILES):
                r0 = t_i * TILE_ROWS  # first token row of this tile
                # --- gather featT tile: rows k*64+c, k-pairs per 128-chunk ---
                fts = []
                for j in range(KC):
                    kr = min(128, INF // C - 2 * j) * C // C * 64
                    kr = 128 if 2 * j + 1 < 25 else 64
                    ft = fpool.tile([kr, TT], f32r, tag=f"f{j}")
                    fts.append(ft)
                    for half in range(kr // 64):
                        k = 2 * j + half
                        dh, dw, q = KPOS[k]
                        src = xe_q[q, :, 1 + r0 + dh : 1 + r0 + dh + TILE_ROWS,
                                   1 + dw : 1 + dw + WIMG]
                        dst = ft[half * 64 : (half + 1) * 64, :].rearrange(
                            "p (r w) -> p r w", r=TILE_ROWS)
                        nc.sync.dma_start(out=dst, in_=src)
                # --- fc1 + GELU: h[m] = gelu(w1p[:,m].T @ featT + b1) ---
                hts = []
                for m in range(HID // 128):
                    ps = ps1.tile([128, TT], f32)
                    for j in range(KC):
                        nc.tensor.matmul(
                            ps[:, :],
                            w1sb[j][:, m * 128 : (m + 1) * 128],
                            fts[j][:, :],
                            start=(j == 0), stop=(j == KC - 1),
                        )
                    ht = hpool.tile([128, TT], f32r, tag=f"h{m}")
                    nc.scalar.activation(ht[:, :], ps[:, :], AF.Gelu,
                                         bias=b1t[:, m : m + 1], scale=1.0)
                    hts.append(ht)
                # --- fc2: out[tok, :] = h.T @ w2t + b2 ---
                for s in range(TT // 128):
                    po = ps2.tile([128, OUTF], f32)
                    for j in range(HID // 128):
                        nc.tensor.matmul(
                            po[:, 0:512],
                            hts[j][:, s * 128 : (s + 1) * 128],
                            w2sb[j][:, 0:512],
                            start=(j == 0), stop=(j == HID // 128 - 1),
                        )
                    for j in range(HID // 128):
                        nc.tensor.matmul(
                            po[:, 512:OUTF],
                            hts[j][:, s * 128 : (s + 1) * 128],
                            w2sb[j][:, 512:OUTF],
                            start=(j == 0), stop=(j == HID // 128 - 1),
                        )
                    ot = opool.tile([128, OUTF], f32, tag="o")
                    nc.vector.tensor_tensor(
                        out=ot[:, :], in0=po[:, :], in1=b2t[:, :], op=Alu.add)
                    tok0 = (r0 * WIMG) + s * 128
                    nc.sync.dma_start(out=out[tok0 : tok0 + 128, :], in_=ot[:, :])

    from wait_split import split_waits
    split_waits(nc)
    _NC_CACHE["nc"] = nc
    return nc


def kernel(x, w1, b1, w2, b2, image_h, image_w):
    x = np.ascontiguousarray(np.asarray(x, dtype=np.float32))
    w1 = np.asarray(w1, dtype=np.float32)
    b1 = np.asarray(b1, dtype=np.float32)
    w2 = np.asarray(w2, dtype=np.float32)
    b2 = np.asarray(b2, dtype=np.float32)

    xe = _build_xe(x)
    w1t = np.ascontiguousarray(w1.T)  # (1600, 1024) rows c*25+k
    w1p = np.ascontiguousarray(
        w1t.reshape(C, 25, HID).transpose(1, 0, 2).reshape(INF, HID))
    w2t = np.ascontiguousarray(w2.T)
    b1rs = np.ascontiguousarray(b1.reshape(HID // 128, 128).T)
    b2bc = np.ascontiguousarray(np.broadcast_to(b2, (128, OUTF)))

    nc = _build_bass()
    in_maps = []
    for cid in range(N_CORES):
        b, half = cid // 2, cid % 2
        h0 = half * ROWS_PER_CORE
        xe_core = np.ascontiguousarray(xe[b, :, h0 : h0 + ROWS_PER_CORE + 2, :])
        in_maps.append({
            "xe": xe_core, "w1p": w1p, "w2t": w2t, "b1rs": b1rs, "b2bc": b2bc,
        })

    from concourse.bass_utils import run_bass_kernel_spmd
    res = run_bass_kernel_spmd(nc, in_maps, list(range(N_CORES)))

    out = np.empty((B, NTOK, OUTF), dtype=np.float32)
    for cid in range(N_CORES):
        b, half = cid // 2, cid % 2
        t0 = half * TOK_PER_CORE
        out[b, t0 : t0 + TOK_PER_CORE, :] = res.results[cid]["out"]
    return out


# revision 7
# speedup vs baseline: 1.2003x; 1.2003x over previous
"""Trainium2 Bass kernel for nn_DMlp_46823733461564 (dense_mlp).

Computes: token-grid 3x3 masked-neighborhood gather (pixel-shuffle +
reflection-pad + masked unfold, algebraically reduced to a channel-
permuted shifted gather) followed by fc1(1600->1024) + exact GELU +
fc2(1024->576).

Sharding: data-parallel over (batch, image-half) -> 8 cores, 8192 tokens
each; fc weights replicated. The gather runs on-device as strided DMAs
from a host-prepared reflection-extended channel-major image; matmuls run
in fp32r (single-pass reduced-precision fp32) on the PE.
"""
import os
import sys

import numpy as np

_TRN_REPO = "/opt/trn_rl_repo"
if _TRN_REPO not in sys.path:
    sys.path.insert(0, _TRN_REPO)

B, HIMG, WIMG = 4, 128, 128
C = 64
L = 576           # C * 9
NTOK = HIMG * WIMG
HID = 1024
OUTF = 576
INF = 1600        # C * 25
N_CORES = 8
ROWS_PER_CORE = HIMG // 2          # 64 token rows
TOK_PER_CORE = ROWS_PER_CORE * WIMG  # 8192
TILE_ROWS = 4                      # image rows per token tile
TT = TILE_ROWS * WIMG              # 512 tokens per tile
N_TILES = ROWS_PER_CORE // TILE_ROWS  # 16
KC = 13                            # ceil(25/2) K-chunks of (up to) 128

_MASK = np.array([
    [1, 0, 0, 1, 0, 0, 1],
    [0, 1, 0, 1, 0, 1, 0],
    [0, 0, 1, 1, 1, 0, 0],
    [1, 1, 1, 1, 1, 1, 1],
    [0, 0, 1, 1, 1, 0, 0],
    [0, 1, 0, 1, 0, 1, 0],
    [1, 0, 0, 1, 0, 0, 1]], dtype=bool)
MASK_POS = [(i, j) for i in range(7) for j in range(7) if _MASK[i, j]]


def _dmap(d):
    if d <= 1:
        return -1, d + 1
    if d <= 4:
        return 0, d - 2
    return 1, d - 5


KPOS = []
for (_di, _dj) in MASK_POS:
    _dh, _r1 = _dmap(_di)
    _dw, _r2 = _dmap(_dj)
    KPOS.append((_dh, _dw, _r1 * 3 + _r2))


def _swap_map(a, b, which):
    ch = np.arange(L)
    c, rem = ch // 9, ch % 9
    r1, r2 = rem // 3, rem % 3
    r = r1 if which == 0 else r2
    rs = np.where(r == a, b, np.where(r == b, a, r))
    if which == 0:
        return c * 9 + rs * 3 + r2
    return c * 9 + r1 * 3 + rs


def _build_xe(x):
    """x: (B, NTOK, L) -> xe: (B, L, HIMG+2, WIMG+2) reflection-extended,
    channel-permuted borders."""
    xt = np.ascontiguousarray(x.transpose(0, 2, 1)).reshape(B, L, HIMG, WIMG)
    xe = np.empty((B, L, HIMG + 2, WIMG + 2), dtype=np.float32)
    xe[:, :, 1:-1, 1:-1] = xt
    xe[:, :, 0, 1:-1] = xt[:, _swap_map(1, 2, 0), 0, :]
    xe[:, :, -1, 1:-1] = xt[:, _swap_map(0, 1, 0), -1, :]
    xe[:, :, :, 0] = np.take(xe[:, :, :, 1], _swap_map(1, 2, 1), axis=1)
    xe[:, :, :, -1] = np.take(xe[:, :, :, -2], _swap_map(0, 1, 1), axis=1)
    return xe


_NC_CACHE = {}

MODE = os.environ.get("KERNEL_DTYPE", "f32r")  # "f32r" | "bf16"


def _build_bass():
    if "nc" in _NC_CACHE:
        return _NC_CACHE["nc"]
    import concourse.bass as bass
    import concourse.mybir as mybir
    from concourse.tile import TileContext

    f32 = mybir.dt.float32
    mm_dt = mybir.dt.float32r if MODE == "f32r" else mybir.dt.bfloat16
    AF = mybir.ActivationFunctionType
    Alu = mybir.AluOpType

    nc = bass.Bass("TRN2", target_bir_lowering=False, debug=False)
    xe = nc.dram_tensor("xe", (L, ROWS_PER_CORE + 2, WIMG + 2), mm_dt,
                        kind="ExternalInput")
    w1p = nc.dram_tensor("w1p", (INF, HID), mm_dt, kind="ExternalInput")
    w2t = nc.dram_tensor("w2t", (HID, OUTF), mm_dt, kind="ExternalInput")
    b1rs = nc.dram_tensor("b1rs", (128, HID // 128), f32, kind="ExternalInput")
    b2bc = nc.dram_tensor("b2bc", (128, OUTF), f32, kind="ExternalInput")
    out = nc.dram_tensor("out", (TOK_PER_CORE, OUTF), f32,
                         kind="ExternalOutput")

    # view of xe with the q sub-pixel index split out: [q, c, rows, cols]
    xe_q = xe.rearrange("(c q) h w -> q c h w", q=9)

    with TileContext(nc) as tc:
        with (
            tc.tile_pool(name="wpool", bufs=1) as wpool,
            tc.tile_pool(name="fpool", bufs=2) as fpool,
            tc.tile_pool(name="hpool", bufs=2) as hpool,
            tc.tile_pool(name="opool", bufs=3) as opool,
            tc.tile_pool(name="ps1", bufs=2, space="PSUM") as ps1,
            tc.tile_pool(name="ps2", bufs=2, space="PSUM") as ps2,
            tc.tile_pool(name="psw", bufs=1, space="PSUM") as psw,
        ):
            # --- PE warmup: dependency-free matmuls fill the initial DMA
            # wait and push the HAM clock gate to 8/8 before real work ---
            n_warm = int(os.environ.get("KERNEL_WARMUP", "72"))
            if n_warm:
                warm = wpool.tile([128, 512], mm_dt, tag="warm")
                nc.any.memset(warm[:, :], 0.0)
                wps = psw.tile([128, 512], f32)
                for _ in range(n_warm):
                    nc.tensor.matmul(wps[:, :], warm[:, 0:128], warm[:, :],
                                     start=True, stop=True)
            # --- replicated weights, loaded once ---
            w1sb = []
            for j in range(KC):
                kr = min(128, INF - j * 128)
                t = wpool.tile([kr, HID], mm_dt, tag=f"w1_{j}")
                nc.sync.dma_start(out=t[:, :], in_=w1p[j * 128 : j * 128 + kr, :])
                w1sb.append(t)
            w2sb = []
            for j in range(HID // 128):
                t = wpool.tile([128, OUTF], mm_dt, tag=f"w2_{j}")
                nc.sync.dma_start(out=t[:, :], in_=w2t[j * 128 : (j + 1) * 128, :])
                w2sb.append(t)
            b1t = wpool.tile([128, HID // 128], f32, tag="b1")
            nc.sync.dma_start(out=b1t[:, :], in_=b1rs[:, :])
            b2t = wpool.tile([128, OUTF], f32, tag="b2")
            nc.sync.dma_start(out=b2t[:, :], in_=b2bc[:, :])

            for t_i in range(N_TILES):
                r0 = t_i * TILE_ROWS  # first token row of this tile
                # --- gather featT tile: rows k*64+c, k-pairs per 128-chunk ---
                fts = []
                for j in range(KC):
                    kr = min(128, INF // C - 2 * j) * C // C * 64
                    kr = 128 if 2 * j + 1 < 25 else 64
                    ft = fpool.tile([kr, TT], mm_dt, tag=f"f{j}")
                    fts.append(ft)
                    for half in range(kr // 64):
                        k = 2 * j + half
                        dh, dw, q = KPOS[k]
                        src = xe_q[q, :, 1 + r0 + dh : 1 + r0 + dh + TILE_ROWS,
                                   1 + dw : 1 + dw + WIMG]
                        dst = ft[half * 64 : (half + 1) * 64, :].rearrange(
                            "p (r w) -> p r w", r=TILE_ROWS)
                        nc.sync.dma_start(out=dst, in_=src)
                # --- fc1 + GELU: h[m] = gelu(w1p[:,m].T @ featT + b1) ---
                hts = []
                for m in range(HID // 128):
                    ps = ps1.tile([128, TT], f32)
                    for j in range(KC):
                        nc.tensor.matmul(
                            ps[:, :],
                            w1sb[j][:, m * 128 : (m + 1) * 128],
                            fts[j][:, :],
                            start=(j == 0), stop=(j == KC - 1),
                        )
                    ht = hpool.tile([128, TT], mm_dt, tag=f"h{m}")
                    nc.scalar.activation(ht[:, :], ps[:, :], AF.Gelu,
                                         bias=b1t[:, m : m + 1], scale=1.0)
                    hts.append(ht)
                # --- fc2: out[tok, :] = h.T @ w2t + b2 ---
                # N split 288+288 so both matmuls stream well past the
                # (hidden) LDWEIGHTS; each [128, 288] psum is one bank.
                NH = OUTF // 2
                for s in range(TT // 128):
                    poa = ps2.tile([128, NH], f32, tag="poa")
                    pob = ps2.tile([128, NH], f32, tag="pob")
                    for j in range(HID // 128):
                        nc.tensor.matmul(
                            poa[:, :],
                            hts[j][:, s * 128 : (s + 1) * 128],
                            w2sb[j][:, 0:NH],
                            start=(j == 0), stop=(j == HID // 128 - 1),
                        )
                        nc.tensor.matmul(
                            pob[:, :],
                            hts[j][:, s * 128 : (s + 1) * 128],
                            w2sb[j][:, NH:OUTF],
                            start=(j == 0), stop=(j == HID // 128 - 1),
                        )
                    ot = opool.tile([128, OUTF], f32, tag="o")
                    nc.vector.tensor_tensor(
                        out=ot[:, 0:NH], in0=poa[:, :], in1=b2t[:, 0:NH],
                        op=Alu.add)
                    nc.vector.tensor_tensor(
                        out=ot[:, NH:OUTF], in0=pob[:, :], in1=b2t[:, NH:OUTF],
                        op=Alu.add)
                    tok0 = (r0 * WIMG) + s * 128
                    nc.sync.dma_start(out=out[tok0 : tok0 + 128, :], in_=ot[:, :])

    from wait_split import split_waits
    split_waits(nc)
    _NC_CACHE["nc"] = nc
    return nc


def _host_prep(x, w1, b1, w2, b2):
    x = np.ascontiguousarray(np.asarray(x, dtype=np.float32))
    w1 = np.asarray(w1, dtype=np.float32)
    b1 = np.asarray(b1, dtype=np.float32)
    w2 = np.asarray(w2, dtype=np.float32)
    b2 = np.asarray(b2, dtype=np.float32)

    xe = _build_xe(x)
    w1t = np.ascontiguousarray(w1.T)  # (1600, 1024) rows c*25+k
    w1p = np.ascontiguousarray(
        w1t.reshape(C, 25, HID).transpose(1, 0, 2).reshape(INF, HID))
    w2t = np.ascontiguousarray(w2.T)
    b1rs = np.ascontiguousarray(b1.reshape(HID // 128, 128).T)
    b2bc = np.ascontiguousarray(np.broadcast_to(b2, (128, OUTF)))

    if MODE == "bf16":
        import ml_dtypes
        xe = xe.astype(ml_dtypes.bfloat16)
        w1p = w1p.astype(ml_dtypes.bfloat16)
        w2t = w2t.astype(ml_dtypes.bfloat16)

    in_maps = []
    for cid in range(N_CORES):
        b, half = cid // 2, cid % 2
        h0 = half * ROWS_PER_CORE
        xe_core = np.ascontiguousarray(xe[b, :, h0 : h0 + ROWS_PER_CORE + 2, :])
        in_maps.append({
            "xe": xe_core, "w1p": w1p, "w2t": w2t, "b1rs": b1rs, "b2bc": b2bc,
        })
    return in_maps


def _assemble(results):
    out = np.empty((B, NTOK, OUTF), dtype=np.float32)
    for cid in range(N_CORES):
        b, half = cid // 2, cid % 2
        t0 = half * TOK_PER_CORE
        out[b, t0 : t0 + TOK_PER_CORE, :] = results[cid]["out"]
    return out


def kernel(x, w1, b1, w2, b2, image_h, image_w):
    in_maps = _host_prep(x, w1, b1, w2, b2)
    nc = _build_bass()
    from concourse.bass_utils import run_bass_kernel_spmd
    res = run_bass_kernel_spmd(nc, in_maps, list(range(N_CORES)))
    return _assemble(res.results)


# revision 11
# speedup vs baseline: 1.2018x; 1.0013x over previous
"""Trainium2 Bass kernel for nn_DMlp_46823733461564 (dense_mlp).

Computes: token-grid 3x3 masked-neighborhood gather (pixel-shuffle +
reflection-pad + masked unfold, algebraically reduced to a channel-
permuted shifted gather) followed by fc1(1600->1024) + exact GELU +
fc2(1024->576).

Sharding: data-parallel over (batch, image-half) -> 8 cores, 8192 tokens
each; fc weights replicated. The gather runs on-device as strided DMAs
from a host-prepared reflection-extended channel-major image; matmuls run
in fp32r (single-pass reduced-precision fp32) on the PE.
"""
import os
import sys

import numpy as np

_TRN_REPO = "/opt/trn_rl_repo"
if _TRN_REPO not in sys.path:
    sys.path.insert(0, _TRN_REPO)

B, HIMG, WIMG = 4, 128, 128
C = 64
L = 576           # C * 9
NTOK = HIMG * WIMG
HID = 1024
OUTF = 576
INF = 1600        # C * 25
N_CORES = 8
ROWS_PER_CORE = HIMG // 2          # 64 token rows
TOK_PER_CORE = ROWS_PER_CORE * WIMG  # 8192
TILE_ROWS = 4                      # image rows per token tile
TT = TILE_ROWS * WIMG              # 512 tokens per tile
N_TILES = ROWS_PER_CORE // TILE_ROWS  # 16
KC = 13                            # ceil(25/2) K-chunks of (up to) 128

_MASK = np.array([
    [1, 0, 0, 1, 0, 0, 1],
    [0, 1, 0, 1, 0, 1, 0],
    [0, 0, 1, 1, 1, 0, 0],
    [1, 1, 1, 1, 1, 1, 1],
    [0, 0, 1, 1, 1, 0, 0],
    [0, 1, 0, 1, 0, 1, 0],
    [1, 0, 0, 1, 0, 0, 1]], dtype=bool)
MASK_POS = [(i, j) for i in range(7) for j in range(7) if _MASK[i, j]]


def _dmap(d):
    if d <= 1:
        return -1, d + 1
    if d <= 4:
        return 0, d - 2
    return 1, d - 5


KPOS = []
for (_di, _dj) in MASK_POS:
    _dh, _r1 = _dmap(_di)
    _dw, _r2 = _dmap(_dj)
    KPOS.append((_dh, _dw, _r1 * 3 + _r2))


def _swap_map(a, b, which):
    ch = np.arange(L)
    c, rem = ch // 9, ch % 9
    r1, r2 = rem // 3, rem % 3
    r = r1 if which == 0 else r2
    rs = np.where(r == a, b, np.where(r == b, a, r))
    if which == 0:
        return c * 9 + rs * 3 + r2
    return c * 9 + r1 * 3 + rs


def _build_xe(x):
    """x: (B, NTOK, L) -> xe: (B, L, HIMG+2, WIMG+2) reflection-extended,
    channel-permuted borders."""
    xt = np.ascontiguousarray(x.transpose(0, 2, 1)).reshape(B, L, HIMG, WIMG)
    xe = np.empty((B, L, HIMG + 2, WIMG + 2), dtype=np.float32)
    xe[:, :, 1:-1, 1:-1] = xt
    xe[:, :, 0, 1:-1] = xt[:, _swap_map(1, 2, 0), 0, :]
    xe[:, :, -1, 1:-1] = xt[:, _swap_map(0, 1, 0), -1, :]
    xe[:, :, :, 0] = np.take(xe[:, :, :, 1], _swap_map(1, 2, 1), axis=1)
    xe[:, :, :, -1] = np.take(xe[:, :, :, -2], _swap_map(0, 1, 1), axis=1)
    return xe


_NC_CACHE = {}

MODE = os.environ.get("KERNEL_DTYPE", "f32r")  # "f32r" | "bf16"


def _build_bass():
    if "nc" in _NC_CACHE:
        return _NC_CACHE["nc"]
    import concourse.bass as bass
    import concourse.mybir as mybir
    from concourse.tile import TileContext

    f32 = mybir.dt.float32
    mm_dt = mybir.dt.float32r if MODE == "f32r" else mybir.dt.bfloat16
    AF = mybir.ActivationFunctionType
    Alu = mybir.AluOpType

    nc = bass.Bass("TRN2", target_bir_lowering=False, debug=False)
    xe = nc.dram_tensor("xe", (L, ROWS_PER_CORE + 2, WIMG + 2), mm_dt,
                        kind="ExternalInput")
    w1p = nc.dram_tensor("w1p", (INF, HID), mm_dt, kind="ExternalInput")
    w2t = nc.dram_tensor("w2t", (HID, OUTF), mm_dt, kind="ExternalInput")
    b1rs = nc.dram_tensor("b1rs", (128, HID // 128), f32, kind="ExternalInput")
    b2bc = nc.dram_tensor("b2bc", (128, OUTF), f32, kind="ExternalInput")
    out = nc.dram_tensor("out", (TOK_PER_CORE, OUTF), f32,
                         kind="ExternalOutput")

    # view of xe with the q sub-pixel index split out: [q, c, rows, cols]
    xe_q = xe.rearrange("(c q) h w -> q c h w", q=9)

    with TileContext(nc) as tc:
        with (
            tc.tile_pool(name="wpool", bufs=1) as wpool,
            tc.tile_pool(name="fpool", bufs=2) as fpool,
            tc.tile_pool(name="hpool", bufs=2) as hpool,
            tc.tile_pool(name="opool", bufs=3) as opool,
            tc.tile_pool(name="ps1", bufs=2, space="PSUM") as ps1,
            tc.tile_pool(name="ps2", bufs=2, space="PSUM") as ps2,
        ):
            pack64 = os.environ.get("KERNEL_PACK64", "0") == "1"
            # --- PE warmup: dependency-free matmuls fill the initial DMA
            # wait and push the HAM clock gate to 8/8 before real work ---
            n_warm = int(os.environ.get("KERNEL_WARMUP", "72"))
            if n_warm:
                warm = wpool.tile([128, 512], mm_dt, tag="warm")
                nc.any.memset(warm[:, :], 0.0)
                wps = ps2.tile([128, 512], f32, tag="poa")
                for _ in range(n_warm):
                    nc.tensor.matmul(wps[:, :], warm[:, 0:128], warm[:, :],
                                     start=True, stop=True)
            # --- replicated weights, loaded once ---
            w1sb = []
            for j in range(KC):
                kr = min(128, INF - j * 128)
                if kr < 128 and pack64:
                    # duplicate the K=64 tail into partitions 64:128 so the
                    # packed row-group matmul can read lhsT from there
                    t = wpool.tile([128, HID], mm_dt, tag=f"w1_{j}")
                    nc.sync.dma_start(out=t[0:kr, :], in_=w1p[j * 128 :, :])
                    nc.sync.dma_start(out=t[kr:128, :], in_=w1p[j * 128 :, :])
                else:
                    t = wpool.tile([kr, HID], mm_dt, tag=f"w1_{j}")
                    nc.sync.dma_start(out=t[:, :], in_=w1p[j * 128 : j * 128 + kr, :])
                w1sb.append(t)
            w2sb = []
            for j in range(HID // 128):
                t = wpool.tile([128, OUTF], mm_dt, tag=f"w2_{j}")
                nc.sync.dma_start(out=t[:, :], in_=w2t[j * 128 : (j + 1) * 128, :])
                w2sb.append(t)
            b1t = wpool.tile([128, HID // 128], f32, tag="b1")
            nc.sync.dma_start(out=b1t[:, :], in_=b1rs[:, :])
            b2t = wpool.tile([128, OUTF], f32, tag="b2")
            nc.sync.dma_start(out=b2t[:, :], in_=b2bc[:, :])

            for t_i in range(N_TILES):
                r0 = t_i * TILE_ROWS  # first token row of this tile
                # --- gather featT tile: rows k*64+c, k-pairs per 128-chunk ---
                fts = []
                for j in range(KC):
                    nk = 2 if 2 * j + 1 < 25 else 1
                    kr = 128 if (nk == 2 or pack64) else 64
                    ft = fpool.tile([kr, TT], mm_dt, tag=f"f{j}")
                    fts.append(ft)
                    for half in range(kr // 64):
                        k = min(2 * j + half, 24)
                        dh, dw, q = KPOS[k]
                        src = xe_q[q, :, 1 + r0 + dh : 1 + r0 + dh + TILE_ROWS,
                                   1 + dw : 1 + dw + WIMG]
                        dst = ft[half * 64 : (half + 1) * 64, :].rearrange(
                            "p (r w) -> p r w", r=TILE_ROWS)
                        nc.sync.dma_start(out=dst, in_=src)
                # --- fc1 + GELU: h[m] = gelu(w1p[:,m].T @ featT + b1) ---
                hts = []
                if not pack64:
                    for m in range(HID // 128):
                        ps = ps1.tile([128, TT], f32)
                        for j in range(KC):
                            nc.tensor.matmul(
                                ps[:, :],
                                w1sb[j][:, m * 128 : (m + 1) * 128],
                                fts[j][:, :],
                                start=(j == 0), stop=(j == KC - 1),
                            )
                        ht = hpool.tile([128, TT], mm_dt, tag=f"h{m}")
                        nc.scalar.activation(ht[:, :], ps[:, :], AF.Gelu,
                                             bias=b1t[:, m : m + 1], scale=1.0)
                        hts.append(ht)
                else:
                    # chunk 12 (K=64) packed: m-pairs run their K=64 matmuls
                    # concurrently on PE row groups (0,0)/(64,0)
                    for mp in range(HID // 256):
                        m0, m1 = 2 * mp, 2 * mp + 1
                        psa = ps1.tile([128, TT], f32, tag="psa")
                        psb = ps1.tile([128, TT], f32, tag="psb")
                        for j in range(KC - 1):
                            nc.tensor.matmul(
                                psa[:, :], w1sb[j][:, m0 * 128:(m0 + 1) * 128],
                                fts[j][:, :], start=(j == 0), stop=False)
                            nc.tensor.matmul(
                                psb[:, :], w1sb[j][:, m1 * 128:(m1 + 1) * 128],
                                fts[j][:, :], start=(j == 0), stop=False)
                        nc.tensor.matmul(
                            psa[:, :], w1sb[KC - 1][0:64, m0 * 128:(m0 + 1) * 128],
                            fts[KC - 1][0:64, :], start=False, stop=True)
                        nc.tensor.matmul(
                            psb[:, :], w1sb[KC - 1][64:128, m1 * 128:(m1 + 1) * 128],
                            fts[KC - 1][64:128, :], start=False, stop=True)
                        for m, pst in ((m0, psa), (m1, psb)):
                            ht = hpool.tile([128, TT], mm_dt, tag=f"h{m}")
                            nc.scalar.activation(ht[:, :], pst[:, :], AF.Gelu,
                                                 bias=b1t[:, m : m + 1], scale=1.0)
                            hts.append(ht)
                # --- fc2: out[tok, :] = h.T @ w2t + b2 ---
                # N split 288+288 so both matmuls stream well past the
                # (hidden) LDWEIGHTS; each [128, 288] psum is one bank.
                NH = OUTF // 2
                for s in range(TT // 128):
                    poa = ps2.tile([128, NH], f32, tag="poa")
                    pob = ps2.tile([128, NH], f32, tag="pob")
                    for j in range(HID // 128):
                        nc.tensor.matmul(
                            poa[:, :],
                            hts[j][:, s * 128 : (s + 1) * 128],
                            w2sb[j][:, 0:NH],
                            start=(j == 0), stop=(j == HID // 128 - 1),
                        )
                        nc.tensor.matmul(
                            pob[:, :],
                            hts[j][:, s * 128 : (s + 1) * 128],
                            w2sb[j][:, NH:OUTF],
                            start=(j == 0), stop=(j == HID // 128 - 1),
                        )
                    ot = opool.tile([128, OUTF], f32, tag="o")
                    nc.vector.tensor_tensor(
                        out=ot[:, 0:NH], in0=poa[:, :], in1=b2t[:, 0:NH],
                        op=Alu.add)
                    nc.vector.tensor_tensor(
                        out=ot[:, NH:OUTF], in0=pob[:, :], in1=b2t[:, NH:OUTF],
                        op=Alu.add)
                    tok0 = (r0 * WIMG) + s * 128
                    nc.sync.dma_start(out=out[tok0 : tok0 + 128, :], in_=ot[:, :])

    from wait_split import split_waits
    split_waits(nc)
    _NC_CACHE["nc"] = nc
    return nc


def _host_prep(x, w1, b1, w2, b2):
    x = np.ascontiguousarray(np.asarray(x, dtype=np.float32))
    w1 = np.asarray(w1, dtype=np.float32)
    b1 = np.asarray(b1, dtype=np.float32)
    w2 = np.asarray(w2, dtype=np.float32)
    b2 = np.asarray(b2, dtype=np.float32)

    xe = _build_xe(x)
    w1t = np.ascontiguousarray(w1.T)  # (1600, 1024) rows c*25+k
    w1p = np.ascontiguousarray(
        w1t.reshape(C, 25, HID).transpose(1, 0, 2).reshape(INF, HID))
    w2t = np.ascontiguousarray(w2.T)
    b1rs = np.ascontiguousarray(b1.reshape(HID // 128, 128).T)
    b2bc = np.ascontiguousarray(np.broadcast_to(b2, (128, OUTF)))

    if MODE == "bf16":
        import ml_dtypes
        xe = xe.astype(ml_dtypes.bfloat16)
        w1p = w1p.astype(ml_dtypes.bfloat16)
        w2t = w2t.astype(ml_dtypes.bfloat16)

    in_maps = []
    for cid in range(N_CORES):
        b, half = cid // 2, cid % 2
        h0 = half * ROWS_PER_CORE
        xe_core = np.ascontiguousarray(xe[b, :, h0 : h0 + ROWS_PER_CORE + 2, :])
        in_maps.append({
            "xe": xe_core, "w1p": w1p, "w2t": w2t, "b1rs": b1rs, "b2bc": b2bc,
        })
    return in_maps


def _assemble(results):
    out = np.empty((B, NTOK, OUTF), dtype=np.float32)
    for cid in range(N_CORES):
        b, half = cid // 2, cid % 2
        t0 = half * TOK_PER_CORE
        out[b, t0 : t0 + TOK_PER_CORE, :] = results[cid]["out"]
    return out


def kernel(x, w1, b1, w2, b2, image_h, image_w):
    in_maps = _host_prep(x, w1, b1, w2, b2)
    nc = _build_bass()
    from concourse.bass_utils import run_bass_kernel_spmd
    res = run_bass_kernel_spmd(nc, in_maps, list(range(N_CORES)))
    return _assemble(res.results)


# revision 13
# speedup vs baseline: 1.2697x; 1.0565x over previous
"""Trainium2 Bass kernel for nn_DMlp_46823733461564 (dense_mlp).

Computes: token-grid 3x3 masked-neighborhood gather (pixel-shuffle +
reflection-pad + masked unfold, algebraically reduced to a channel-
permuted shifted gather) followed by fc1(1600->1024) + exact GELU +
fc2(1024->576).

Sharding: data-parallel over (batch, image-half) -> 8 cores, 8192 tokens
each; fc weights replicated. The gather runs on-device as strided DMAs
from a host-prepared reflection-extended channel-major image; matmuls run
in fp32r (single-pass reduced-precision fp32) on the PE.
"""
import os
import sys

import numpy as np

_TRN_REPO = "/opt/trn_rl_repo"
if _TRN_REPO not in sys.path:
    sys.path.insert(0, _TRN_REPO)

B, HIMG, WIMG = 4, 128, 128
C = 64
L = 576           # C * 9
NTOK = HIMG * WIMG
HID = 1024
OUTF = 576
INF = 1600        # C * 25
N_CORES = 8
ROWS_PER_CORE = HIMG // 2          # 64 token rows
TOK_PER_CORE = ROWS_PER_CORE * WIMG  # 8192
TILE_ROWS = 4                      # image rows per token tile
TT = TILE_ROWS * WIMG              # 512 tokens per tile
N_TILES = ROWS_PER_CORE // TILE_ROWS  # 16
KC = 13                            # ceil(25/2) K-chunks of (up to) 128

_MASK = np.array([
    [1, 0, 0, 1, 0, 0, 1],
    [0, 1, 0, 1, 0, 1, 0],
    [0, 0, 1, 1, 1, 0, 0],
    [1, 1, 1, 1, 1, 1, 1],
    [0, 0, 1, 1, 1, 0, 0],
    [0, 1, 0, 1, 0, 1, 0],
    [1, 0, 0, 1, 0, 0, 1]], dtype=bool)
MASK_POS = [(i, j) for i in range(7) for j in range(7) if _MASK[i, j]]


def _dmap(d):
    if d <= 1:
        return -1, d + 1
    if d <= 4:
        return 0, d - 2
    return 1, d - 5


KPOS = []
for (_di, _dj) in MASK_POS:
    _dh, _r1 = _dmap(_di)
    _dw, _r2 = _dmap(_dj)
    KPOS.append((_dh, _dw, _r1 * 3 + _r2))


def _swap_map(a, b, which):
    ch = np.arange(L)
    c, rem = ch // 9, ch % 9
    r1, r2 = rem // 3, rem % 3
    r = r1 if which == 0 else r2
    rs = np.where(r == a, b, np.where(r == b, a, r))
    if which == 0:
        return c * 9 + rs * 3 + r2
    return c * 9 + r1 * 3 + rs


def _build_xe(x):
    """x: (B, NTOK, L) -> xe: (B, L, HIMG+2, WIMG+2) reflection-extended,
    channel-permuted borders."""
    xt = np.ascontiguousarray(x.transpose(0, 2, 1)).reshape(B, L, HIMG, WIMG)
    xe = np.empty((B, L, HIMG + 2, WIMG + 2), dtype=np.float32)
    xe[:, :, 1:-1, 1:-1] = xt
    xe[:, :, 0, 1:-1] = xt[:, _swap_map(1, 2, 0), 0, :]
    xe[:, :, -1, 1:-1] = xt[:, _swap_map(0, 1, 0), -1, :]
    xe[:, :, :, 0] = np.take(xe[:, :, :, 1], _swap_map(1, 2, 1), axis=1)
    xe[:, :, :, -1] = np.take(xe[:, :, :, -2], _swap_map(0, 1, 1), axis=1)
    return xe


_NC_CACHE = {}

MODE = os.environ.get("KERNEL_DTYPE", "f32r")  # "f32r" | "bf16"


def _build_bass():
    if "nc" in _NC_CACHE:
        return _NC_CACHE["nc"]
    import concourse.bass as bass
    import concourse.mybir as mybir
    from concourse.tile import TileContext

    f32 = mybir.dt.float32
    mm_dt = mybir.dt.float32r if MODE == "f32r" else mybir.dt.bfloat16
    AF = mybir.ActivationFunctionType
    Alu = mybir.AluOpType

    nc = bass.Bass("TRN2", target_bir_lowering=False, debug=False)
    xe = nc.dram_tensor("xe", (L, ROWS_PER_CORE + 2, WIMG + 2), mm_dt,
                        kind="ExternalInput")
    w1p = nc.dram_tensor("w1p", (INF, HID), mm_dt, kind="ExternalInput")
    w2t = nc.dram_tensor("w2t", (HID, OUTF), mm_dt, kind="ExternalInput")
    b1rs = nc.dram_tensor("b1rs", (128, HID // 128), f32, kind="ExternalInput")
    b2bc = nc.dram_tensor("b2bc", (128, OUTF), f32, kind="ExternalInput")
    out = nc.dram_tensor("out", (TOK_PER_CORE, OUTF), f32,
                         kind="ExternalOutput")

    # view of xe with the q sub-pixel index split out: [q, c, rows, cols]
    xe_q = xe.rearrange("(c q) h w -> q c h w", q=9)

    with TileContext(nc) as tc:
        with (
            tc.tile_pool(name="wpool", bufs=1) as wpool,
            tc.tile_pool(name="fpool", bufs=2) as fpool,
            tc.tile_pool(name="hpool", bufs=2) as hpool,
            tc.tile_pool(name="opool", bufs=3) as opool,
            tc.tile_pool(name="ps1", bufs=2, space="PSUM") as ps1,
            tc.tile_pool(name="ps2", bufs=2, space="PSUM") as ps2,
        ):
            pack64 = os.environ.get("KERNEL_PACK64", "0") == "1"
            # --- PE warmup: dependency-free matmuls fill the initial DMA
            # wait and push the HAM clock gate to 8/8 before real work ---
            n_warm = int(os.environ.get("KERNEL_WARMUP", "72"))
            if n_warm:
                warm = wpool.tile([128, 512], mm_dt, tag="warm")
                nc.any.memset(warm[:, :], 0.0)
                wps = ps2.tile([128, 512], f32, tag="poa")
                for _ in range(n_warm):
                    nc.tensor.matmul(wps[:, :], warm[:, 0:128], warm[:, :],
                                     start=True, stop=True)
            # --- replicated weights, loaded once ---
            w1sb = []
            for j in range(KC):
                kr = min(128, INF - j * 128)
                if kr < 128 and pack64:
                    # duplicate the K=64 tail into partitions 64:128 so the
                    # packed row-group matmul can read lhsT from there
                    t = wpool.tile([128, HID], mm_dt, tag=f"w1_{j}")
                    nc.sync.dma_start(out=t[0:kr, :], in_=w1p[j * 128 :, :])
                    nc.sync.dma_start(out=t[kr:128, :], in_=w1p[j * 128 :, :])
                else:
                    t = wpool.tile([kr, HID], mm_dt, tag=f"w1_{j}")
                    nc.sync.dma_start(out=t[:, :], in_=w1p[j * 128 : j * 128 + kr, :])
                w1sb.append(t)
            w2sb = []
            for j in range(HID // 128):
                t = wpool.tile([128, OUTF], mm_dt, tag=f"w2_{j}")
                nc.sync.dma_start(out=t[:, :], in_=w2t[j * 128 : (j + 1) * 128, :])
                w2sb.append(t)
            b1t = wpool.tile([128, HID // 128], f32, tag="b1")
            nc.sync.dma_start(out=b1t[:, :], in_=b1rs[:, :])
            b2t = wpool.tile([128, OUTF], f32, tag="b2")
            nc.sync.dma_start(out=b2t[:, :], in_=b2bc[:, :])

            def emit_fc2(hts, r0):
                # --- fc2: out[tok, :] = h.T @ w2t + b2 ---
                # N split 288+288 so both matmuls stream well past the
                # (hidden) LDWEIGHTS; each [128, 288] psum is one bank.
                NH = OUTF // 2
                for s in range(TT // 128):
                    poa = ps2.tile([128, NH], f32, tag="poa")
                    pob = ps2.tile([128, NH], f32, tag="pob")
                    for j in range(HID // 128):
                        nc.tensor.matmul(
                            poa[:, :],
                            hts[j][:, s * 128 : (s + 1) * 128],
                            w2sb[j][:, 0:NH],
                            start=(j == 0), stop=(j == HID // 128 - 1),
                        )
                        nc.tensor.matmul(
                            pob[:, :],
                            hts[j][:, s * 128 : (s + 1) * 128],
                            w2sb[j][:, NH:OUTF],
                            start=(j == 0), stop=(j == HID // 128 - 1),
                        )
                    ot = opool.tile([128, OUTF], f32, tag="o")
                    nc.vector.tensor_tensor(
                        out=ot[:, 0:NH], in0=poa[:, :], in1=b2t[:, 0:NH],
                        op=Alu.add)
                    nc.vector.tensor_tensor(
                        out=ot[:, NH:OUTF], in0=pob[:, :], in1=b2t[:, NH:OUTF],
                        op=Alu.add)
                    tok0 = (r0 * WIMG) + s * 128
                    nc.sync.dma_start(out=out[tok0 : tok0 + 128, :], in_=ot[:, :])

            prev = None  # (hts, r0) of the previous tile: fc2 runs one
            # tile behind fc1 so the PE never waits on the GELU latency
            for t_i in range(N_TILES):
                r0 = t_i * TILE_ROWS  # first token row of this tile
                # --- gather featT tile: rows k*64+c, k-pairs per 128-chunk ---
                fts = []
                for j in range(KC):
                    nk = 2 if 2 * j + 1 < 25 else 1
                    kr = 128 if (nk == 2 or pack64) else 64
                    ft = fpool.tile([kr, TT], mm_dt, tag=f"f{j}")
                    fts.append(ft)
                    for half in range(kr // 64):
                        k = min(2 * j + half, 24)
                        dh, dw, q = KPOS[k]
                        src = xe_q[q, :, 1 + r0 + dh : 1 + r0 + dh + TILE_ROWS,
                                   1 + dw : 1 + dw + WIMG]
                        dst = ft[half * 64 : (half + 1) * 64, :].rearrange(
                            "p (r w) -> p r w", r=TILE_ROWS)
                        nc.sync.dma_start(out=dst, in_=src)
                # --- fc1 + GELU: h[m] = gelu(w1p[:,m].T @ featT + b1) ---
                hts = []
                if not pack64:
                    for m in range(HID // 128):
                        ps = ps1.tile([128, TT], f32)
                        for j in range(KC):
                            nc.tensor.matmul(
                                ps[:, :],
                                w1sb[j][:, m * 128 : (m + 1) * 128],
                                fts[j][:, :],
                                start=(j == 0), stop=(j == KC - 1),
                            )
                        ht = hpool.tile([128, TT], mm_dt, tag=f"h{m}")
                        nc.scalar.activation(ht[:, :], ps[:, :], AF.Gelu,
                                             bias=b1t[:, m : m + 1], scale=1.0)
                        hts.append(ht)
                else:
                    # chunk 12 (K=64) packed: m-pairs run their K=64 matmuls
                    # concurrently on PE row groups (0,0)/(64,0)
                    for mp in range(HID // 256):
                        m0, m1 = 2 * mp, 2 * mp + 1
                        psa = ps1.tile([128, TT], f32, tag="psa")
                        psb = ps1.tile([128, TT], f32, tag="psb")
                        for j in range(KC - 1):
                            nc.tensor.matmul(
                                psa[:, :], w1sb[j][:, m0 * 128:(m0 + 1) * 128],
                                fts[j][:, :], start=(j == 0), stop=False)
                            nc.tensor.matmul(
                                psb[:, :], w1sb[j][:, m1 * 128:(m1 + 1) * 128],
                                fts[j][:, :], start=(j == 0), stop=False)
                        nc.tensor.matmul(
                            psa[:, :], w1sb[KC - 1][0:64, m0 * 128:(m0 + 1) * 128],
                            fts[KC - 1][0:64, :], start=False, stop=True)
                        nc.tensor.matmul(
                            psb[:, :], w1sb[KC - 1][64:128, m1 * 128:(m1 + 1) * 128],
                            fts[KC - 1][64:128, :], start=False, stop=True)
                        for m, pst in ((m0, psa), (m1, psb)):
                            ht = hpool.tile([128, TT], mm_dt, tag=f"h{m}")
                            nc.scalar.activation(ht[:, :], pst[:, :], AF.Gelu,
                                                 bias=b1t[:, m : m + 1], scale=1.0)
                            hts.append(ht)
                if prev is not None:
                    emit_fc2(*prev)
                prev = (hts, r0)
            emit_fc2(*prev)

    from wait_split import split_waits
    split_waits(nc)
    _NC_CACHE["nc"] = nc
    return nc


def _host_prep(x, w1, b1, w2, b2):
    x = np.ascontiguousarray(np.asarray(x, dtype=np.float32))
    w1 = np.asarray(w1, dtype=np.float32)
    b1 = np.asarray(b1, dtype=np.float32)
    w2 = np.asarray(w2, dtype=np.float32)
    b2 = np.asarray(b2, dtype=np.float32)

    xe = _build_xe(x)
    w1t = np.ascontiguousarray(w1.T)  # (1600, 1024) rows c*25+k
    w1p = np.ascontiguousarray(
        w1t.reshape(C, 25, HID).transpose(1, 0, 2).reshape(INF, HID))
    w2t = np.ascontiguousarray(w2.T)
    b1rs = np.ascontiguousarray(b1.reshape(HID // 128, 128).T)
    b2bc = np.ascontiguousarray(np.broadcast_to(b2, (128, OUTF)))

    if MODE == "bf16":
        import ml_dtypes
        xe = xe.astype(ml_dtypes.bfloat16)
        w1p = w1p.astype(ml_dtypes.bfloat16)
        w2t = w2t.astype(ml_dtypes.bfloat16)

    in_maps = []
    for cid in range(N_CORES):
        b, half = cid // 2, cid % 2
        h0 = half * ROWS_PER_CORE
        xe_core = np.ascontiguousarray(xe[b, :, h0 : h0 + ROWS_PER_CORE + 2, :])
        in_maps.append({
            "xe": xe_core, "w1p": w1p, "w2t": w2t, "b1rs": b1rs, "b2bc": b2bc,
        })
    return in_maps


def _assemble(results):
    out = np.empty((B, NTOK, OUTF), dtype=np.float32)
    for cid in range(N_CORES):
        b, half = cid // 2, cid % 2
        t0 = half * TOK_PER_CORE
        out[b, t0 : t0 + TOK_PER_CORE, :] = results[cid]["out"]
    return out


def kernel(x, w1, b1, w2, b2, image_h, image_w):
    in_maps = _host_prep(x, w1, b1, w2, b2)
    nc = _build_bass()
    from concourse.bass_utils import run_bass_kernel_spmd
    res = run_bass_kernel_spmd(nc, in_maps, list(range(N_CORES)))
    return _assemble(res.results)
